# revision 6
# baseline (speedup 1.0000x reference)
"""Trainium2 Bass kernel: additive (Bahdanau-style) attention.

Reference math (B=16, Lq=Lc=H=256):
    qp  = query @ Wq.T                  (B, Lq, H)
    cp  = context @ Wc.T                (B, Lc, H)
    x   = qp[:,:,None,:] + cp[:,None,:,:] + w_bias     (B, Lq, Lc, H)
    score = leaky_relu(x) @ v           (B, Lq, Lc)
    score = where(mask==0, -inf, score)
    attn = softmax(score, -1); attn_output = attn @ context
    returns (attn_output, attn)

Device strategy (8 NeuronCores, data-parallel over batch, 2 batches/core):
  leaky(x) = s*x + (1-s)*relu(x), s=0.01. So
    score[q,c] = s*(vq[q] + vc[c] + mask_bias[c]/s...) handled as:
      - relu part: for each (q, htile): X = relu(cpT + qpbT[:,q])   (128h, 256c)
        computed on VectorE (fused tensor_scalar add+max, bf16) and ScalarE
        (activation Relu with per-partition bias), then reduced over h by the
        TensorEngine using a sliding one-hot stationary tile W99 where
        lhsT = W99[:, 128-j:256-j] == 0.99*v (x) e_j^T, which accumulates q's
        score row directly onto PSUM partition j of a (128, 256) S tile.
      - linear part + mask: one K=2 rank-2 matmul into the same PSUM
        accumulation group: S += outer(0.01*vq, ones) + outer(ones, rowvec)
        with rowvec = 0.01*vc + (mask-1)*1e30.
  softmax: reduce_max (negated) -> Exp activation with bias=-max and
  accum_out row-sum -> reciprocal -> scale. attn_output via PE transpose of
  attn and bf16 matmuls against context.
"""

import os
import numpy as np
from contextlib import ExitStack

import concourse.bass as bass
import concourse.mybir as mybir
import concourse.tile as tile
from concourse import bacc
from concourse.bass_utils import run_bass_kernel_spmd
from concourse.masks import make_identity

F32 = mybir.dt.float32
BF16 = mybir.dt.bfloat16
I32 = mybir.dt.int32
AF = mybir.ActivationFunctionType
OP = mybir.AluOpType
AX = mybir.AxisListType

B, L, H = 16, 256, 256
NCORES = 8
BL = B // NCORES          # batches per core
P = 128                   # partitions
HT = H // P               # h tiles (contraction side)
QT = L // P               # q tiles
CT = L // P               # c tiles
SLOPE = 0.01
ACT_EVERY = 6             # every Nth elementwise op runs on ScalarE


def _build_body(ctx, tc):
    nc = tc.nc
    q_d = nc.declare_dram_parameter("query", [BL, L, H], F32, isOutput=False)
    c_d = nc.declare_dram_parameter("context", [BL, L, H], F32, isOutput=False)
    m_d = nc.declare_dram_parameter("mask", [BL, L], I32, isOutput=False)
    w_d = nc.declare_dram_parameter("w_weight", [H, 2 * H], F32, isOutput=False)
    b_d = nc.declare_dram_parameter("w_bias", [H], F32, isOutput=False)
    v_d = nc.declare_dram_parameter("score_weight", [1, H], F32, isOutput=False)
    ao_d = nc.declare_dram_parameter("attn_output", [BL, L, H], F32, isOutput=True)
    at_d = nc.declare_dram_parameter("attn", [BL, L, L], F32, isOutput=True)

    consts = ctx.enter_context(tc.tile_pool(name="consts", bufs=1))
    wpool = ctx.enter_context(tc.tile_pool(name="wpool", bufs=1))
    bpool = ctx.enter_context(tc.tile_pool(name="bpool", bufs=4))
    xpool = ctx.enter_context(tc.tile_pool(name="xpool", bufs=8))
    spool = ctx.enter_context(tc.tile_pool(name="spool", bufs=4))
    psA = ctx.enter_context(tc.tile_pool(name="psA", bufs=3, space="PSUM"))
    psB = ctx.enter_context(tc.tile_pool(name="psB", bufs=4, space="PSUM"))

    # ---------------- constants / weights (once) ----------------
    ident = consts.tile([P, P], F32)
    make_identity(nc, ident)
    ones_row = consts.tile([1, L], F32)
    nc.vector.memset(ones_row, 1.0)

    wsb = []
    for r in range(HT):  # rows r*P:(r+1)*P of w_weight (h_out)
        t = wpool.tile([P, 2 * H], F32, tag=f"wsb{r}")
        nc.sync.dma_start(out=t, in_=w_d[r * P:(r + 1) * P, :])
        wsb.append(t)

    vrow = wpool.tile([1, H], F32, tag="vrow")
    nc.sync.dma_start(out=vrow, in_=v_d[0:1, :])
    brow = wpool.tile([1, H], F32, tag="brow")
    nc.sync.dma_start(out=brow, in_=b_d[None, :])

    # transposed weights: wqT[ki], wcT[ki] are (h_in tile ki on partitions, h_out on free)
    wqT = [wpool.tile([P, H], F32, tag=f"wqT{k}", name=f"wqT{k}") for k in range(HT)]
    wcT = [wpool.tile([P, H], F32, tag=f"wcT{k}", name=f"wcT{k}") for k in range(HT)]
    for ki in range(HT):
        for r in range(HT):
            for dst, coff in ((wqT, 0), (wcT, H)):
                pst = psB.tile([P, P], F32, tag="ps")
                nc.tensor.transpose(pst, wsb[r][:, coff + ki * P: coff + (ki + 1) * P], ident)
                nc.scalar.copy(out=dst[ki][:, r * P:(r + 1) * P], in_=pst)

    # v and bias as per-partition columns
    vcolf, bcol, vcol16, w99 = [], [], [], []
    for ht in range(HT):
        pv = psB.tile([P, 1], F32, tag="ps")
        nc.tensor.transpose(pv, vrow[0:1, ht * P:(ht + 1) * P], ident[0:1, 0:1])
        t = wpool.tile([P, 1], F32, tag=f"vcolf{ht}")
        nc.vector.tensor_copy(out=t, in_=pv)
        vcolf.append(t)

        pb = psB.tile([P, 1], F32, tag="ps")
        nc.tensor.transpose(pb, brow[0:1, ht * P:(ht + 1) * P], ident[0:1, 0:1])
        tb = wpool.tile([P, 1], F32, tag=f"bcol{ht}")
        nc.vector.tensor_copy(out=tb, in_=pb)
        bcol.append(tb)

        t16 = wpool.tile([P, 1], BF16, tag=f"vcol16{ht}")
        nc.vector.tensor_copy(out=t16, in_=pv)
        vcol16.append(t16)

        # sliding one-hot stationary: zero except column P = (1-s)*v_half
        tw = wpool.tile([P, 2 * P], BF16, tag=f"w99{ht}")
        nc.vector.memset(tw, 0.0)
        nc.vector.tensor_scalar(out=tw[:, P:P + 1], in0=pv, scalar1=1.0 - SLOPE,
                                scalar2=None, op0=OP.mult)
        w99.append(tw)

    # ---------------- per batch ----------------
    for b in range(BL):
        qsb = [bpool.tile([P, H], F32, tag=f"qsb{i}", name=f"qsb{i}") for i in range(QT)]
        csb = [bpool.tile([P, H], F32, tag=f"csb{i}", name=f"csb{i}") for i in range(CT)]
        for qi in range(QT):
            nc.sync.dma_start(out=qsb[qi], in_=q_d[b, qi * P:(qi + 1) * P, :])
        for ci in range(CT):
            nc.sync.dma_start(out=csb[ci], in_=c_d[b, ci * P:(ci + 1) * P, :])
        csb16 = [bpool.tile([P, H], BF16, tag=f"csb16{i}", name=f"csb16{i}") for i in range(CT)]
        for ci in range(CT):
            nc.vector.tensor_copy(out=csb16[ci], in_=csb[ci])

        mrow_i = bpool.tile([1, L], I32, tag="mrow_i")
        nc.sync.dma_start(out=mrow_i, in_=m_d[b:b + 1, :])
        mrow_f = bpool.tile([1, L], F32, tag="mrow_f")
        nc.vector.tensor_copy(out=mrow_f, in_=mrow_i)
        maskb = bpool.tile([1, L], F32, tag="maskb")
        nc.vector.tensor_scalar(out=maskb, in0=mrow_f, scalar1=-1.0, scalar2=1e30,
                                op0=OP.add, op1=OP.mult)

        # transposes: qT/cT = (h on partitions, q/c on free)
        qT = [bpool.tile([P, L], F32, tag=f"qT{i}", name=f"qT{i}") for i in range(HT)]
        cT = [bpool.tile([P, L], F32, tag=f"cT{i}", name=f"cT{i}") for i in range(HT)]
        for src, dst in ((qsb, qT), (csb, cT)):
            for ti in range(QT):
                for hi in range(HT):
                    pst = psB.tile([P, P], F32, tag="ps")
                    nc.tensor.transpose(pst, src[ti][:, hi * P:(hi + 1) * P], ident)
                    nc.scalar.copy(out=dst[hi][:, ti * P:(ti + 1) * P], in_=pst)

        # projections
        qpbT = [bpool.tile([P, L], F32, tag=f"qpbT{i}", name=f"qpbT{i}") for i in range(HT)]
        cpT16 = [bpool.tile([P, L], BF16, tag=f"cpT16{i}", name=f"cpT16{i}") for i in range(HT)]
        for ho in range(HT):
            pq = psB.tile([P, L], F32, tag="ps")
            for ki in range(HT):
                nc.tensor.matmul(pq, wqT[ki][:, ho * P:(ho + 1) * P], qT[ki],
                                 start=(ki == 0), stop=(ki == HT - 1))
            nc.vector.tensor_scalar(out=qpbT[ho], in0=pq, scalar1=bcol[ho],
                                    scalar2=None, op0=OP.add)
            pc = psB.tile([P, L], F32, tag="ps")
            for ki in range(HT):
                nc.tensor.matmul(pc, wcT[ki][:, ho * P:(ho + 1) * P], cT[ki],
                                 start=(ki == 0), stop=(ki == HT - 1))
            nc.scalar.copy(out=cpT16[ho], in_=pc)

        # vq / vc rows via M=1 matvecs (land on psum partition 0)
        pvq = psB.tile([1, L], F32, tag="ps")
        for ht in range(HT):
            nc.tensor.matmul(pvq, vcolf[ht], qpbT[ht], start=(ht == 0), stop=(ht == HT - 1))
        pvc = psB.tile([1, L], F32, tag="ps")
        for ht in range(HT):
            nc.tensor.matmul(pvc, vcol16[ht], cpT16[ht], start=(ht == 0), stop=(ht == HT - 1))

        linq = bpool.tile([1, L], F32, tag="linq")
        nc.vector.tensor_scalar(out=linq, in0=pvq, scalar1=SLOPE,
                                scalar2=None, op0=OP.mult)
        rowvec = bpool.tile([1, L], F32, tag="rowvec")
        nc.vector.tensor_scalar(out=rowvec, in0=pvc, scalar1=SLOPE,
                                scalar2=None, op0=OP.mult)
        nc.vector.tensor_add(rowvec, rowvec, maskb)

        attnT16 = [bpool.tile([P, L], BF16, tag=f"attnT16{i}", name=f"attnT16{i}") for i in range(CT)]

        opctr = 0
        for qi in range(QT):
            sp = psA.tile([P, L], F32, tag="spsum")
            nc.tensor.matmul(sp, linq[0:1, qi * P:(qi + 1) * P], ones_row,
                             start=True, stop=False)
            nc.tensor.matmul(sp, ones_row[0:1, 0:P], rowvec,
                             start=False, stop=False)
            for j in range(P):
                q = qi * P + j
                for ht in range(HT):
                    x = xpool.tile([P, L], BF16, tag="x")
                    bias_col = qpbT[ht][:, q:q + 1]
                    if opctr % ACT_EVERY == ACT_EVERY - 1:
                        nc.scalar.activation(out=x, in_=cpT16[ht], func=AF.Relu,
                                             bias=bias_col, scale=1.0)
                    else:
                        nc.vector.tensor_scalar(out=x, in0=cpT16[ht], scalar1=bias_col,
                                                scalar2=0.0, op0=OP.add, op1=OP.max)
                    opctr += 1
                    last = (j == P - 1 and ht == HT - 1)
                    nc.tensor.matmul(sp, w99[ht][:, P - j:2 * P - j], x,
                                     start=False, stop=last)

            # softmax over c
            mx = spool.tile([P, 1], F32, tag="mx")
            nc.vector.tensor_reduce(out=mx, in_=sp, axis=AX.X, op=OP.max, negate=True)
            pexp = spool.tile([P, L], F32, tag="pexp")
            rsum = spool.tile([P, 1], F32, tag="rsum")
            nc.scalar.activation(out=pexp, in_=sp, func=AF.Exp, bias=mx, scale=1.0,
                                 accum_out=rsum)
            rinv = spool.tile([P, 1], F32, tag="rinv")
            nc.vector.reciprocal(out=rinv, in_=rsum)
            attn_sb = spool.tile([P, L], F32, tag="attn_sb")
            nc.vector.tensor_scalar(out=attn_sb, in0=pexp, scalar1=rinv,
                                    scalar2=None, op0=OP.mult)
            nc.sync.dma_start(out=at_d[b, qi * P:(qi + 1) * P, :], in_=attn_sb)

            # attn_output = attn @ context
            for ci in range(CT):
                pst = psB.tile([P, P], F32, tag="ps")
                nc.tensor.transpose(pst, attn_sb[:, ci * P:(ci + 1) * P], ident)
                nc.vector.tensor_copy(out=attnT16[ci][:, qi * P:(qi + 1) * P], in_=pst)
            po = psB.tile([P, H], F32, tag="ps")
            for ci in range(CT):
                nc.tensor.matmul(po, attnT16[ci][:, qi * P:(qi + 1) * P], csb16[ci],
                                 start=(ci == 0), stop=(ci == CT - 1))
            osb = spool.tile([P, H], F32, tag="osb")
            nc.scalar.copy(out=osb, in_=po)
            nc.sync.dma_start(out=ao_d[b, qi * P:(qi + 1) * P, :], in_=osb)


_NC_CACHE = {}


def build_nc():
    if "nc" in _NC_CACHE:
        return _NC_CACHE["nc"]
    nc = bacc.Bacc("TRN2", target_bir_lowering=False)
    with ExitStack() as ctx:
        tc = ctx.enter_context(tile.TileContext(nc))
        _build_body(ctx, tc)
    nc.compile()
    _NC_CACHE["nc"] = nc
    return nc


def kernel(query, context, mask, w_weight, w_bias, score_weight, _trace=False):
    query = np.ascontiguousarray(np.asarray(query, dtype=np.float32))
    context = np.ascontiguousarray(np.asarray(context, dtype=np.float32))
    mask = np.ascontiguousarray(np.asarray(mask, dtype=np.int32))
    w_weight = np.ascontiguousarray(np.asarray(w_weight, dtype=np.float32))
    w_bias = np.ascontiguousarray(np.asarray(w_bias, dtype=np.float32))
    score_weight = np.ascontiguousarray(np.asarray(score_weight, dtype=np.float32))

    nc = build_nc()
    in_maps = []
    for i in range(NCORES):
        sl = slice(i * BL, (i + 1) * BL)
        in_maps.append({
            "query": query[sl], "context": context[sl], "mask": mask[sl],
            "w_weight": w_weight, "w_bias": w_bias, "score_weight": score_weight,
        })
    res = run_bass_kernel_spmd(nc, in_maps, core_ids=list(range(NCORES)),
                               trace=_trace)
    attn_output = np.concatenate([r["attn_output"] for r in res.results], axis=0)
    attn = np.concatenate([r["attn"] for r in res.results], axis=0)
    if _trace:
        kernel.last_exec_time_ns = res.exec_time_ns
        kernel.last_results = res
    return attn_output, attn


# revision 8
# speedup vs baseline: 1.0363x; 1.0363x over previous
"""Trainium2 Bass kernel: additive (Bahdanau-style) attention.

Reference math (B=16, Lq=Lc=H=256):
    qp  = query @ Wq.T                  (B, Lq, H)
    cp  = context @ Wc.T                (B, Lc, H)
    x   = qp[:,:,None,:] + cp[:,None,:,:] + w_bias     (B, Lq, Lc, H)
    score = leaky_relu(x) @ v           (B, Lq, Lc)
    score = where(mask==0, -inf, score)
    attn = softmax(score, -1); attn_output = attn @ context
    returns (attn_output, attn)

Device strategy (8 NeuronCores, data-parallel over batch, 2 batches/core):
  leaky(x) = s*x + (1-s)*relu(x), s=0.01. So
    score[q,c] = s*(vq[q] + vc[c] + mask_bias[c]/s...) handled as:
      - relu part: for each (q, htile): X = relu(cpT + qpbT[:,q])   (128h, 256c)
        computed on VectorE (fused tensor_scalar add+max, bf16) and ScalarE
        (activation Relu with per-partition bias), then reduced over h by the
        TensorEngine using a sliding one-hot stationary tile W99 where
        lhsT = W99[:, 128-j:256-j] == 0.99*v (x) e_j^T, which accumulates q's
        score row directly onto PSUM partition j of a (128, 256) S tile.
      - linear part + mask: one K=2 rank-2 matmul into the same PSUM
        accumulation group: S += outer(0.01*vq, ones) + outer(ones, rowvec)
        with rowvec = 0.01*vc + (mask-1)*1e30.
  softmax: reduce_max (negated) -> Exp activation with bias=-max and
  accum_out row-sum -> reciprocal -> scale. attn_output via PE transpose of
  attn and bf16 matmuls against context.
"""

import os
import numpy as np
from contextlib import ExitStack

import concourse.bass as bass
import concourse.mybir as mybir
import concourse.tile as tile
from concourse import bacc
from concourse.bass_utils import run_bass_kernel_spmd
from concourse.masks import make_identity

F32 = mybir.dt.float32
BF16 = mybir.dt.bfloat16
I32 = mybir.dt.int32
AF = mybir.ActivationFunctionType
OP = mybir.AluOpType
AX = mybir.AxisListType

B, L, H = 16, 256, 256
NCORES = 8
BL = B // NCORES          # batches per core
P = 128                   # partitions
HT = H // P               # h tiles (contraction side)
QT = L // P               # q tiles
CT = L // P               # c tiles
SLOPE = 0.01
ACT_EVERY = 6             # every Nth elementwise op runs on ScalarE


def _build_body(ctx, tc):
    nc = tc.nc
    q_d = nc.declare_dram_parameter("query", [BL, L, H], F32, isOutput=False)
    c_d = nc.declare_dram_parameter("context", [BL, L, H], F32, isOutput=False)
    m_d = nc.declare_dram_parameter("mask", [BL, L], I32, isOutput=False)
    w_d = nc.declare_dram_parameter("w_weight", [H, 2 * H], F32, isOutput=False)
    b_d = nc.declare_dram_parameter("w_bias", [H], F32, isOutput=False)
    v_d = nc.declare_dram_parameter("score_weight", [1, H], F32, isOutput=False)
    ao_d = nc.declare_dram_parameter("attn_output", [BL, L, H], F32, isOutput=True)
    at_d = nc.declare_dram_parameter("attn", [BL, L, L], F32, isOutput=True)

    consts = ctx.enter_context(tc.tile_pool(name="consts", bufs=1))
    wpool = ctx.enter_context(tc.tile_pool(name="wpool", bufs=1))
    bpool = ctx.enter_context(tc.tile_pool(name="bpool", bufs=4))
    xpool = ctx.enter_context(tc.tile_pool(name="xpool", bufs=8))
    spool = ctx.enter_context(tc.tile_pool(name="spool", bufs=4))
    psA = ctx.enter_context(tc.tile_pool(name="psA", bufs=3, space="PSUM"))
    psB = ctx.enter_context(tc.tile_pool(name="psB", bufs=4, space="PSUM"))

    # ---------------- constants / weights (once) ----------------
    ident = consts.tile([P, P], F32)
    make_identity(nc, ident)
    ones_row = consts.tile([1, L], F32)
    nc.vector.memset(ones_row, 1.0)

    wsb = []
    for r in range(HT):  # rows r*P:(r+1)*P of w_weight (h_out)
        t = wpool.tile([P, 2 * H], F32, tag=f"wsb{r}")
        nc.sync.dma_start(out=t, in_=w_d[r * P:(r + 1) * P, :])
        wsb.append(t)

    vrow = wpool.tile([1, H], F32, tag="vrow")
    nc.sync.dma_start(out=vrow, in_=v_d[0:1, :])
    brow = wpool.tile([1, H], F32, tag="brow")
    nc.sync.dma_start(out=brow, in_=b_d[None, :])

    # transposed weights: wqT[ki], wcT[ki] are (h_in tile ki on partitions, h_out on free)
    wqT = [wpool.tile([P, H], F32, tag=f"wqT{k}", name=f"wqT{k}") for k in range(HT)]
    wcT = [wpool.tile([P, H], F32, tag=f"wcT{k}", name=f"wcT{k}") for k in range(HT)]
    for ki in range(HT):
        for r in range(HT):
            for dst, coff in ((wqT, 0), (wcT, H)):
                pst = psB.tile([P, P], F32, tag="ps")
                nc.tensor.transpose(pst, wsb[r][:, coff + ki * P: coff + (ki + 1) * P], ident)
                nc.scalar.copy(out=dst[ki][:, r * P:(r + 1) * P], in_=pst)

    # v and bias as per-partition columns
    vcolf, bcol, vcol16, w99 = [], [], [], []
    for ht in range(HT):
        pv = psB.tile([P, 1], F32, tag="ps")
        nc.tensor.transpose(pv, vrow[0:1, ht * P:(ht + 1) * P], ident[0:1, 0:1])
        t = wpool.tile([P, 1], F32, tag=f"vcolf{ht}")
        nc.vector.tensor_copy(out=t, in_=pv)
        vcolf.append(t)

        pb = psB.tile([P, 1], F32, tag="ps")
        nc.tensor.transpose(pb, brow[0:1, ht * P:(ht + 1) * P], ident[0:1, 0:1])
        tb = wpool.tile([P, 1], F32, tag=f"bcol{ht}")
        nc.vector.tensor_copy(out=tb, in_=pb)
        bcol.append(tb)

        t16 = wpool.tile([P, 1], BF16, tag=f"vcol16{ht}")
        nc.vector.tensor_copy(out=t16, in_=pv)
        vcol16.append(t16)

        # sliding one-hot stationaries for 32-column-group matmuls.
        # we[:, 32] = 0.99*v (used for even j%32), wo[:, 33] = same (odd j%32)
        # so every 32-wide slice [32-jr : 64-jr] / [33-jr : 65-jr] starts at a
        # 4-byte-aligned bf16 offset.
        we = wpool.tile([P, 64], BF16, tag=f"w99e{ht}", name=f"w99e{ht}")
        nc.vector.memset(we, 0.0)
        nc.vector.tensor_scalar(out=we[:, 32:33], in0=pv, scalar1=1.0 - SLOPE,
                                scalar2=None, op0=OP.mult)
        wo = wpool.tile([P, 66], BF16, tag=f"w99o{ht}", name=f"w99o{ht}")
        nc.vector.memset(wo, 0.0)
        nc.vector.tensor_scalar(out=wo[:, 33:34], in0=pv, scalar1=1.0 - SLOPE,
                                scalar2=None, op0=OP.mult)
        w99.append((we, wo))

    # ---------------- per batch ----------------
    for b in range(BL):
        qsb = [bpool.tile([P, H], F32, tag=f"qsb{i}", name=f"qsb{i}") for i in range(QT)]
        csb = [bpool.tile([P, H], F32, tag=f"csb{i}", name=f"csb{i}") for i in range(CT)]
        for qi in range(QT):
            nc.sync.dma_start(out=qsb[qi], in_=q_d[b, qi * P:(qi + 1) * P, :])
        for ci in range(CT):
            nc.sync.dma_start(out=csb[ci], in_=c_d[b, ci * P:(ci + 1) * P, :])
        csb16 = [bpool.tile([P, H], BF16, tag=f"csb16{i}", name=f"csb16{i}") for i in range(CT)]
        for ci in range(CT):
            nc.vector.tensor_copy(out=csb16[ci], in_=csb[ci])

        mrow_i = bpool.tile([1, L], I32, tag="mrow_i")
        nc.sync.dma_start(out=mrow_i, in_=m_d[b:b + 1, :])
        mrow_f = bpool.tile([1, L], F32, tag="mrow_f")
        nc.vector.tensor_copy(out=mrow_f, in_=mrow_i)
        maskb = bpool.tile([1, L], F32, tag="maskb")
        nc.vector.tensor_scalar(out=maskb, in0=mrow_f, scalar1=-1.0, scalar2=1e30,
                                op0=OP.add, op1=OP.mult)

        # transposes: qT/cT = (h on partitions, q/c on free)
        qT = [bpool.tile([P, L], F32, tag=f"qT{i}", name=f"qT{i}") for i in range(HT)]
        cT = [bpool.tile([P, L], F32, tag=f"cT{i}", name=f"cT{i}") for i in range(HT)]
        for src, dst in ((qsb, qT), (csb, cT)):
            for ti in range(QT):
                for hi in range(HT):
                    pst = psB.tile([P, P], F32, tag="ps")
                    nc.tensor.transpose(pst, src[ti][:, hi * P:(hi + 1) * P], ident)
                    nc.scalar.copy(out=dst[hi][:, ti * P:(ti + 1) * P], in_=pst)

        # projections
        qpbT = [bpool.tile([P, L], F32, tag=f"qpbT{i}", name=f"qpbT{i}") for i in range(HT)]
        cpT16 = [bpool.tile([P, L], BF16, tag=f"cpT16{i}", name=f"cpT16{i}") for i in range(HT)]
        for ho in range(HT):
            pq = psB.tile([P, L], F32, tag="ps")
            for ki in range(HT):
                nc.tensor.matmul(pq, wqT[ki][:, ho * P:(ho + 1) * P], qT[ki],
                                 start=(ki == 0), stop=(ki == HT - 1))
            nc.vector.tensor_scalar(out=qpbT[ho], in0=pq, scalar1=bcol[ho],
                                    scalar2=None, op0=OP.add)
            pc = psB.tile([P, L], F32, tag="ps")
            for ki in range(HT):
                nc.tensor.matmul(pc, wcT[ki][:, ho * P:(ho + 1) * P], cT[ki],
                                 start=(ki == 0), stop=(ki == HT - 1))
            nc.scalar.copy(out=cpT16[ho], in_=pc)

        # vq / vc rows via M=1 matvecs (land on psum partition 0)
        pvq = psB.tile([1, L], F32, tag="ps")
        for ht in range(HT):
            nc.tensor.matmul(pvq, vcolf[ht], qpbT[ht], start=(ht == 0), stop=(ht == HT - 1))
        pvc = psB.tile([1, L], F32, tag="ps")
        for ht in range(HT):
            nc.tensor.matmul(pvc, vcol16[ht], cpT16[ht], start=(ht == 0), stop=(ht == HT - 1))

        linq = bpool.tile([1, L], F32, tag="linq")
        nc.vector.tensor_scalar(out=linq, in0=pvq, scalar1=SLOPE,
                                scalar2=None, op0=OP.mult)
        rowvec = bpool.tile([1, L], F32, tag="rowvec")
        nc.vector.tensor_scalar(out=rowvec, in0=pvc, scalar1=SLOPE,
                                scalar2=None, op0=OP.mult)
        nc.vector.tensor_add(rowvec, rowvec, maskb)

        attnT16 = [bpool.tile([P, L], BF16, tag=f"attnT16{i}", name=f"attnT16{i}") for i in range(CT)]

        opctr = 0
        for qi in range(QT):
            sp = psA.tile([P, L], F32, tag="spsum")
            nc.tensor.matmul(sp, linq[0:1, qi * P:(qi + 1) * P], ones_row,
                             start=True, stop=False)
            nc.tensor.matmul(sp, ones_row[0:1, 0:P], rowvec,
                             start=False, stop=False)
            for j in range(P):
                q = qi * P + j
                g, jr = j // 32, j % 32
                for ht in range(HT):
                    x = xpool.tile([P, L], BF16, tag="x")
                    bias_col = qpbT[ht][:, q:q + 1]
                    if opctr % 14 in (4, 9, 13):
                        nc.scalar.activation(out=x, in_=cpT16[ht], func=AF.Relu,
                                             bias=bias_col, scale=1.0)
                    else:
                        nc.vector.tensor_scalar(out=x, in0=cpT16[ht], scalar1=bias_col,
                                                scalar2=0.0, op0=OP.add, op1=OP.max)
                    opctr += 1
                    last = (j == P - 1 and ht == HT - 1)
                    we, wo = w99[ht]
                    lhsT = we[:, 32 - jr:64 - jr] if jr % 2 == 0 else wo[:, 33 - jr:65 - jr]
                    nc.tensor.matmul(sp[32 * g:32 * (g + 1), :], lhsT, x,
                                     start=False, stop=last,
                                     tile_position=(0, 32 * g),
                                     skip_group_check=True)

            # softmax over c
            mx = spool.tile([P, 1], F32, tag="mx")
            nc.vector.tensor_reduce(out=mx, in_=sp, axis=AX.X, op=OP.max, negate=True)
            pexp = spool.tile([P, L], F32, tag="pexp")
            rsum = spool.tile([P, 1], F32, tag="rsum")
            nc.scalar.activation(out=pexp, in_=sp, func=AF.Exp, bias=mx, scale=1.0,
                                 accum_out=rsum)
            rinv = spool.tile([P, 1], F32, tag="rinv")
            nc.vector.reciprocal(out=rinv, in_=rsum)
            attn_sb = spool.tile([P, L], F32, tag="attn_sb")
            nc.vector.tensor_scalar(out=attn_sb, in0=pexp, scalar1=rinv,
                                    scalar2=None, op0=OP.mult)
            nc.sync.dma_start(out=at_d[b, qi * P:(qi + 1) * P, :], in_=attn_sb)

            # attn_output = attn @ context
            for ci in range(CT):
                pst = psB.tile([P, P], F32, tag="ps")
                nc.tensor.transpose(pst, attn_sb[:, ci * P:(ci + 1) * P], ident)
                nc.vector.tensor_copy(out=attnT16[ci][:, qi * P:(qi + 1) * P], in_=pst)
            po = psB.tile([P, H], F32, tag="ps")
            for ci in range(CT):
                nc.tensor.matmul(po, attnT16[ci][:, qi * P:(qi + 1) * P], csb16[ci],
                                 start=(ci == 0), stop=(ci == CT - 1))
            osb = spool.tile([P, H], F32, tag="osb")
            nc.scalar.copy(out=osb, in_=po)
            nc.sync.dma_start(out=ao_d[b, qi * P:(qi + 1) * P, :], in_=osb)


_NC_CACHE = {}


def build_nc():
    if "nc" in _NC_CACHE:
        return _NC_CACHE["nc"]
    nc = bacc.Bacc("TRN2", target_bir_lowering=False)
    with ExitStack() as ctx:
        tc = ctx.enter_context(tile.TileContext(nc))
        _build_body(ctx, tc)
    nc.compile()
    _NC_CACHE["nc"] = nc
    return nc


def kernel(query, context, mask, w_weight, w_bias, score_weight, _trace=False):
    query = np.ascontiguousarray(np.asarray(query, dtype=np.float32))
    context = np.ascontiguousarray(np.asarray(context, dtype=np.float32))
    mask = np.ascontiguousarray(np.asarray(mask, dtype=np.int32))
    w_weight = np.ascontiguousarray(np.asarray(w_weight, dtype=np.float32))
    w_bias = np.ascontiguousarray(np.asarray(w_bias, dtype=np.float32))
    score_weight = np.ascontiguousarray(np.asarray(score_weight, dtype=np.float32))

    nc = build_nc()
    in_maps = []
    for i in range(NCORES):
        sl = slice(i * BL, (i + 1) * BL)
        in_maps.append({
            "query": query[sl], "context": context[sl], "mask": mask[sl],
            "w_weight": w_weight, "w_bias": w_bias, "score_weight": score_weight,
        })
    res = run_bass_kernel_spmd(nc, in_maps, core_ids=list(range(NCORES)),
                               trace=_trace)
    attn_output = np.concatenate([r["attn_output"] for r in res.results], axis=0)
    attn = np.concatenate([r["attn"] for r in res.results], axis=0)
    if _trace:
        kernel.last_exec_time_ns = res.exec_time_ns
        kernel.last_results = res
    return attn_output, attn
